# revision 22
# baseline (speedup 1.0000x reference)
"""Bass/Trainium2 kernel for nn_BaseSmear: project a 64^3 voxel grid into 4
camera images, bilinear-sample 32 channels + depth/validity/view-dir channels.

Sharding: 8 cores = 4 cameras x 2 grid halves (ix in [0,32)/[32,64)).
Each core processes 131072 voxel-camera pairs against one camera image.

Gather: images are pre-packed on the host (fp16) into "x-window" chunks:
chunk key (py, by, bxe) holds rows (2*by+py, +1) x cols (2*bxe .. 2*bxe+3)
x 32 channels = 2*4*32 fp16 = 512B. For a sample at (y0, x0):
  py = y0&1, by = y0>>1, bxe = x0>>1  ->  idx = py*16384 + by*128 + bxe
fits int16 (max 32766) as required by dma_gather. The two x taps sit at
window offset t = u - 2*bxe in [0,2); bilinear becomes a 6-tap blend with
hat-function x-weights xw_k = relu(1 - |t - k|), k = 0..2.
"""

import numpy as np

G = 64
NCAM = 4
C = 32
H = W = 256
NCORES = 8
HALF = 32          # ix slabs per core
P = 128            # partitions
F = HALF * 32      # free cols in full-core tiles; (p, f=(ix,j)) <-> voxel
JS = 32            # j cols per slab; in-slab voxel v = j*128 + p
NV = HALF * G * G  # 131072 voxels per core
CHUNK = 256        # fp16 elems per chunk (2*4*32)
NCHUNK = 32768

_built = None
_perm = None


def _get_perm():
    global _perm
    if _perm is None:
        try:
            import jax
            with jax.default_device(jax.devices('cpu')[0]):
                _perm = np.asarray(jax.random.permutation(jax.random.key(42), NCAM))
        except Exception:
            _perm = np.array([2, 1, 3, 0])  # jax.random.permutation(key(42), 4)
    return _perm


def _pack_images(images):
    """images (4,32,256,256) f32 -> per-cam [32768, 256] fp16 chunk buffers."""
    out = []
    for cam in range(NCAM):
        hwc = np.ascontiguousarray(images[cam].transpose(1, 2, 0))  # (H,W,C)
        pad = np.pad(hwc, ((0, 1), (0, 2), (0, 0)), mode='edge')    # (257,258,C)
        # win[y, x, c, ty, tx] = pad[y+ty, x+tx, c]
        win = np.lib.stride_tricks.sliding_window_view(pad, (2, 4), axis=(0, 1))
        sel = win[:, 0:254 + 1:2]            # x starts 0,2,...,254 -> [256,128,C,2,4]
        sel = sel.transpose(0, 1, 3, 4, 2)   # [y, bxe, ty, tx, c]
        r = sel.reshape(128, 2, 128, 2, 4, C).transpose(1, 0, 2, 3, 4, 5)
        out.append(np.ascontiguousarray(
            r.reshape(NCHUNK, CHUNK), dtype=np.float16))
    return out


def _core_params(cam, h, T_0w, center, pitch, transformations, T_cw):
    """28 affine coefficients: for q in (u_num, v_num, zden, depth, vx, vy, vz):
    value(ix_l, iy, iz) = c0*ix_l + c1*iy + c2*iz + c3."""
    T_0w = np.asarray(T_0w, np.float64)
    center = np.asarray(center, np.float64)
    pitch = float(pitch)
    Tr = np.asarray(transformations, np.float64)[cam]   # (3,4)
    Tc = np.asarray(T_cw, np.float64)[cam]              # (4,4)

    Rm, t = T_0w[:3, :3], T_0w[:3, 3]
    R_w0 = Rm.T
    t_w0 = -(Rm.T @ t)
    Mh = np.zeros((4, 4), np.float64)
    Mh[:3, :3] = R_w0 * pitch
    cst = center + pitch * np.array([h * HALF - (G - 1) / 2.0,
                                     -(G - 1) / 2.0, -(G - 1) / 2.0])
    Mh[:3, 3] = R_w0 @ cst + t_w0
    Mh[3, 3] = 1.0

    A_proj = Tr @ Mh                      # (3,4): u_num, v_num, zden rows
    A_depth = Tc[2, :] @ Mh               # (4,)
    Rc, tc = Tc[:3, :3], Tc[:3, 3]
    cam_c = -(Rc.T @ tc)
    A_view = Mh[:3].copy()
    A_view[:, 3] -= cam_c

    rows = [A_proj[0], A_proj[1], A_proj[2], A_depth,
            A_view[0], A_view[1], A_view[2]]
    par = np.zeros((32,), np.float32)
    for q, r in enumerate(rows):
        par[4 * q:4 * q + 4] = r.astype(np.float32)
    return np.tile(par[None, :], (P, 1))


def _build(debug=False):
    import concourse.bass as bass
    import concourse.mybir as mybir
    from concourse import bacc
    from concourse.tile import TileContext

    dt = mybir.dt
    op = mybir.AluOpType
    AF = mybir.ActivationFunctionType

    nc = bacc.Bacc(trn_type='TRN2')
    imgb = nc.dram_tensor('imgb', [NCHUNK, CHUNK], dt.float16, kind='ExternalInput')
    params = nc.dram_tensor('params', [P, 32], dt.float32, kind='ExternalInput')
    feats_out = nc.dram_tensor('feats', [HALF, P, C * JS], dt.float32,
                               kind='ExternalOutput')
    extras_out = nc.dram_tensor('extras', [5, P, F], dt.float32,
                                kind='ExternalOutput')
    idx_dram = nc.dram_tensor('idx_scratch', [8, HALF, G * G], dt.int16,
                              kind='Internal')
    dbg_out = None
    if debug:
        dbg_out = nc.dram_tensor('dbg', [8, P, F], dt.float32, kind='ExternalOutput')

    with TileContext(nc) as tc:
        f32, f16, i32, i16 = dt.float32, dt.float16, dt.int32, dt.int16
        with tc.tile_pool(name='persist', bufs=1) as pp:

            def PT(name, dtype=f32, cols=F):
                return pp.tile([P, cols], dtype, name=name, tag=name)

            par = pp.tile([P, 32], f32, name='par', tag='par')
            nc.sync.dma_start(out=par[:], in_=params[:])
            b_p1 = pp.tile([P, 1], f32, name='b_p1', tag='b_p1')
            b_m1 = pp.tile([P, 1], f32, name='b_m1', tag='b_m1')
            nc.vector.memset(b_p1[:], 1.0)
            nc.vector.memset(b_m1[:], -1.0)

            def ps(k):
                return par[:, k:k + 1]

            s6 = PT('s6', f32, F * 6)     # packed weights (f, k=(ty*3+tx))
            s6h = PT('s6h', f16, F * 6)
            depth_t = PT('depth_t')
            valid = PT('valid')
            vwx = PT('vwx')
            vwy = PT('vwy')
            vwz = PT('vwz')
            idx16 = PT('idx16', i16)

            sp_ctx = tc.tile_pool(name='scratch', bufs=1)
            sp = sp_ctx.__enter__()

            def T(name, dtype=f32):
                return sp.tile([P, F], dtype, name=name, tag=name)

            # ---- static index tiles: ix = f>>5, j = f&31, iy = 2j + (p>>6),
            # iz = p & 63 ----
            tp_i = T('tp_i', i32)
            tf_i = T('tf_i', i32)
            nc.gpsimd.iota(tp_i[:], pattern=[[0, F]], base=0, channel_multiplier=1)
            nc.gpsimd.iota(tf_i[:], pattern=[[1, F]], base=0, channel_multiplier=0)

            it1 = T('it1', i32)
            fIX = T('fIX')
            fIY = T('fIY')
            fIZ = T('fIZ')
            nc.vector.tensor_scalar(out=it1[:], in0=tf_i[:], scalar1=5, scalar2=None,
                                    op0=op.arith_shift_right)
            nc.vector.tensor_copy(out=fIX[:], in_=it1[:])
            nc.vector.tensor_scalar(out=it1[:], in0=tf_i[:], scalar1=31, scalar2=None,
                                    op0=op.bitwise_and)
            nc.vector.tensor_copy(out=fIY[:], in_=it1[:])  # = j for now
            nc.vector.tensor_scalar(out=it1[:], in0=tp_i[:], scalar1=6, scalar2=None,
                                    op0=op.arith_shift_right)
            nc.gpsimd.tensor_copy(out=fIZ[:], in_=it1[:])  # = p>>6 for now
            # iy = 2*j + (p>>6)
            nc.vector.tensor_scalar(out=fIY[:], in0=fIY[:], scalar1=2.0, scalar2=None,
                                    op0=op.mult)
            nc.vector.tensor_tensor(out=fIY[:], in0=fIY[:], in1=fIZ[:], op=op.add)
            # iz = p & 63
            nc.vector.tensor_scalar(out=it1[:], in0=tp_i[:], scalar1=63, scalar2=None,
                                    op0=op.bitwise_and)
            nc.vector.tensor_copy(out=fIZ[:], in_=it1[:])

            # ---- affine quantities ----
            u_num = T('u_num')
            v_num = T('v_num')
            zden = T('zden')
            aff = {'u_num': u_num, 'v_num': v_num, 'zden': zden,
                   'depth': depth_t, 'vwx': vwx, 'vwy': vwy, 'vwz': vwz}
            names = ['u_num', 'v_num', 'zden', 'depth', 'vwx', 'vwy', 'vwz']
            tmpa = T('tmpa')
            tmpb = T('tmpb')
            for q, nm in enumerate(names):
                t = aff[nm]
                c0, c1, c2, c3 = (ps(4 * q), ps(4 * q + 1), ps(4 * q + 2),
                                  ps(4 * q + 3))
                nc.vector.tensor_scalar(out=t[:], in0=fIX[:], scalar1=c0,
                                        scalar2=None, op0=op.mult)
                nc.vector.tensor_scalar(out=tmpa[:], in0=fIY[:], scalar1=c1,
                                        scalar2=None, op0=op.mult)
                nc.vector.tensor_tensor(out=t[:], in0=t[:], in1=tmpa[:], op=op.add)
                nc.vector.tensor_scalar(out=tmpb[:], in0=fIZ[:], scalar1=c2,
                                        scalar2=c3, op0=op.mult, op1=op.add)
                nc.vector.tensor_tensor(out=t[:], in0=t[:], in1=tmpb[:], op=op.add)

            # ---- projection ----
            rz = T('rz')
            scr = T('scr')
            nc.vector.reciprocal_approx_accurate(out=rz[:], in_=zden[:],
                                                 scratch=scr[:])
            u = T('u')
            v = T('v')
            nc.vector.tensor_tensor(out=u[:], in0=u_num[:], in1=rz[:], op=op.mult)
            nc.vector.tensor_tensor(out=v[:], in0=v_num[:], in1=rz[:], op=op.mult)

            # ---- validity ----
            ta = T('ta')
            tb = T('tb')
            nc.vector.tensor_scalar(out=ta[:], in0=u[:], scalar1=0.0, scalar2=None,
                                    op0=op.is_ge)
            nc.vector.tensor_scalar(out=tb[:], in0=u[:], scalar1=float(W - 1),
                                    scalar2=None, op0=op.is_le)
            nc.vector.tensor_tensor(out=valid[:], in0=ta[:], in1=tb[:], op=op.mult)
            nc.vector.tensor_scalar(out=ta[:], in0=v[:], scalar1=0.0, scalar2=None,
                                    op0=op.is_ge)
            nc.vector.tensor_tensor(out=valid[:], in0=valid[:], in1=ta[:], op=op.mult)
            nc.vector.tensor_scalar(out=tb[:], in0=v[:], scalar1=float(H - 1),
                                    scalar2=None, op0=op.is_le)
            nc.vector.tensor_tensor(out=valid[:], in0=valid[:], in1=tb[:], op=op.mult)
            nc.gpsimd.tensor_scalar(out=ta[:], in0=depth_t[:], scalar1=0.0,
                                    scalar2=None, op0=op.is_gt)
            nc.vector.tensor_tensor(out=valid[:], in0=valid[:], in1=ta[:], op=op.mult)

            # ---- clamped coords ----
            ucl = T('ucl')
            vcl = T('vcl')
            nc.vector.tensor_scalar(out=ucl[:], in0=u[:], scalar1=0.0,
                                    scalar2=float(W - 1), op0=op.max, op1=op.min)
            nc.vector.tensor_scalar(out=vcl[:], in0=v[:], scalar1=0.0,
                                    scalar2=float(H - 1), op0=op.max, op1=op.min)

            # ---- y: y0 = floor(vcl) via round(vcl-0.5); wy = vcl - y0 ----
            y0i = T('y0i', i32)
            y0f = T('y0f')
            wy = T('wy')
            nc.vector.tensor_scalar(out=tmpa[:], in0=vcl[:], scalar1=-0.5,
                                    scalar2=None, op0=op.add)
            nc.vector.tensor_copy(out=y0i[:], in_=tmpa[:])  # round-to-nearest-even
            nc.gpsimd.tensor_copy(out=y0f[:], in_=y0i[:])
            nc.vector.tensor_tensor(out=wy[:], in0=vcl[:], in1=y0f[:], op=op.subtract)

            # ---- x: bxe = floor(ucl/2) via round(ucl*0.5-0.5); t = ucl-2*bxe ----
            bxi = T('bxi', i32)
            bxf = T('bxf')
            tt = T('tt')
            nc.vector.tensor_scalar(out=tmpa[:], in0=ucl[:], scalar1=0.5,
                                    scalar2=-0.5, op0=op.mult, op1=op.add)
            nc.vector.tensor_copy(out=bxi[:], in_=tmpa[:])
            nc.gpsimd.tensor_copy(out=bxf[:], in_=bxi[:])
            nc.vector.tensor_scalar(out=tmpb[:], in0=bxf[:], scalar1=-2.0,
                                    scalar2=None, op0=op.mult)
            nc.vector.tensor_tensor(out=tt[:], in0=ucl[:], in1=tmpb[:], op=op.add)

            # ---- hat x-weights: xw_k = relu(1 - |t - k|) ----
            xw0 = T('xw0')
            xw1 = T('xw1')
            xw2 = T('xw2')
            nc.scalar.activation(out=xw0[:], in_=tt[:], func=AF.Relu,
                                 bias=b_p1[:], scale=-1.0)
            nc.scalar.activation(out=tmpa[:], in_=tt[:], func=AF.Abs,
                                 bias=b_m1[:], scale=1.0)
            nc.scalar.activation(out=xw1[:], in_=tmpa[:], func=AF.Relu,
                                 bias=b_p1[:], scale=-1.0)
            nc.scalar.activation(out=xw2[:], in_=tt[:], func=AF.Relu,
                                 bias=b_m1[:], scale=1.0)

            # ---- y weights x validity ----
            yv0 = T('yv0')
            yv1 = T('yv1')
            nc.vector.tensor_scalar(out=tmpa[:], in0=wy[:], scalar1=-1.0,
                                    scalar2=1.0, op0=op.mult, op1=op.add)
            nc.vector.tensor_tensor(out=yv0[:], in0=valid[:], in1=tmpa[:], op=op.mult)
            nc.vector.tensor_tensor(out=yv1[:], in0=valid[:], in1=wy[:], op=op.mult)

            # ---- packed 6 weights s6[p, f*6 + ty*3 + tx] ----
            s6v = s6[:].rearrange('p (f k) -> p f k', k=6)
            for ty, yv in enumerate([yv0, yv1]):
                for tx, xw in enumerate([xw0, xw1, xw2]):
                    eng = nc.vector if (ty * 3 + tx) % 2 == 0 else nc.gpsimd
                    eng.tensor_tensor(out=s6v[:, :, ty * 3 + tx], in0=yv[:],
                                      in1=xw[:], op=op.mult)
            nc.vector.tensor_copy(out=s6h[:], in_=s6[:])

            # ---- gather idx: idx = py*16384 + by*128 + bxe (int) ----
            iti = T('iti', i32)
            itj = T('itj', i32)
            nc.vector.tensor_scalar(out=iti[:], in0=y0i[:], scalar1=1, scalar2=None,
                                    op0=op.bitwise_and)
            nc.vector.tensor_scalar(out=iti[:], in0=iti[:], scalar1=14, scalar2=None,
                                    op0=op.logical_shift_left)
            nc.vector.tensor_scalar(out=itj[:], in0=y0i[:], scalar1=1, scalar2=None,
                                    op0=op.arith_shift_right)
            nc.vector.tensor_scalar(out=itj[:], in0=itj[:], scalar1=7, scalar2=None,
                                    op0=op.logical_shift_left)
            nc.vector.tensor_tensor(out=iti[:], in0=iti[:], in1=itj[:], op=op.add)
            nc.vector.tensor_tensor(out=iti[:], in0=iti[:], in1=bxi[:], op=op.add)
            nc.vector.tensor_copy(out=idx16[:], in_=iti[:])
            # stage to DRAM in gather order, replicated 8x (one copy per Q7
            # core's 16-partition idx group): dram[r][ix][i = j*128 + p]
            for r in range(8):
                nc.sync.dma_start(
                    out=idx_dram[r].rearrange('x (j p) -> p x j', p=P, j=JS),
                    in_=idx16[:].rearrange('p (x j) -> p x j', x=HALF, j=JS))

            # ---- view directions (normalize) ----
            n2 = T('n2')
            nc.vector.tensor_tensor(out=n2[:], in0=vwx[:], in1=vwx[:], op=op.mult)
            nc.vector.tensor_tensor(out=ta[:], in0=vwy[:], in1=vwy[:], op=op.mult)
            nc.vector.tensor_tensor(out=n2[:], in0=n2[:], in1=ta[:], op=op.add)
            nc.gpsimd.tensor_tensor(out=tb[:], in0=vwz[:], in1=vwz[:], op=op.mult)
            nc.vector.tensor_tensor(out=n2[:], in0=n2[:], in1=tb[:], op=op.add)
            sn = T('sn')
            nc.scalar.sqrt(out=sn[:], in_=n2[:])
            rn = T('rn')
            nc.vector.reciprocal_approx_accurate(out=rn[:], in_=sn[:], scratch=scr[:])
            nc.vector.tensor_tensor(out=vwx[:], in0=vwx[:], in1=rn[:], op=op.mult)
            nc.vector.tensor_tensor(out=vwy[:], in0=vwy[:], in1=rn[:], op=op.mult)
            nc.vector.tensor_tensor(out=vwz[:], in0=vwz[:], in1=rn[:], op=op.mult)

            if debug:
                for k, t in enumerate([u, v, valid, tt, wy, depth_t, zden, bxf]):
                    nc.sync.dma_start(out=dbg_out[k], in_=t[:])

            # ---- extras channels out ----
            for k, t in enumerate([depth_t, valid, vwx, vwy, vwz]):
                nc.sync.dma_start(out=extras_out[k], in_=t[:])

            sp_ctx.__exit__(None, None, None)
            gp_ctx = tc.tile_pool(name='gath', bufs=3)
            mp_ctx = tc.tile_pool(name='mtile', bufs=2)
            outp_ctx = tc.tile_pool(name='outs', bufs=3)
            ip_ctx = tc.tile_pool(name='idxp', bufs=3)
            gp = gp_ctx.__enter__()
            mp = mp_ctx.__enter__()
            outp = outp_ctx.__enter__()
            ip = ip_ctx.__enter__()

            s6hv = s6h[:].rearrange('p (f k) -> p f k', k=6)

            for ix in range(HALF):
                ixt = ip.tile([P, G * G // 16], i16, name='ixt', tag='ixt')
                for r in range(8):
                    nc.sync.dma_start(
                        out=ixt[r * 16:(r + 1) * 16, :],
                        in_=idx_dram[r, ix].rearrange('(c q) -> q c', q=16))
                gt = gp.tile([P, JS * CHUNK], f16, name='gt', tag='g')
                g3 = gt[:].rearrange('p (j e) -> p j e', j=JS, e=CHUNK)
                nc.gpsimd.dma_gather(g3, imgb[:], ixt[:], G * G, G * G, CHUNK,
                                     single_packet=False)

                # W expansion: W[p, (j, k=(ty,tx), c)] = s6h[p, (ix*32+j, k)]
                wt = mp.tile([P, JS * 6 * C], f16, name='wt', tag='wt')
                wv = wt[:].rearrange('p (j k c) -> p j k c', j=JS, k=6, c=C)
                for k in range(6):
                    src = s6hv[:, ix * JS:(ix + 1) * JS, k].unsqueeze(2) \
                        .broadcast_to([P, JS, C])
                    eng = nc.scalar if k % 2 == 0 else nc.gpsimd
                    if eng is nc.scalar:
                        eng.activation(out=wv[:, :, k], in_=src,
                                       func=mybir.ActivationFunctionType.Copy)
                    else:
                        eng.tensor_copy(out=wv[:, :, k], in_=src)

                # M = G_taps * W  (taps: ty in {0,1} x tx in {0,1,2})
                mt = mp.tile([P, JS * 6 * C], f16, name='mt', tag='mt')
                mv = mt[:].rearrange('p (j k c) -> p j k c', j=JS, k=6, c=C)
                g4 = gt[:].rearrange('p (j ty tx c) -> p j ty tx c',
                                     j=JS, ty=2, tx=4, c=C)
                gsel = g4[:, :, :, 0:3]  # [p, j, ty, tx(3), c]
                mv4 = mt[:].rearrange('p (j ty tx c) -> p j ty tx c',
                                      j=JS, ty=2, tx=3, c=C)
                wv4 = wt[:].rearrange('p (j ty tx c) -> p j ty tx c',
                                      j=JS, ty=2, tx=3, c=C)
                nc.vector.tensor_tensor(out=mv4, in0=gsel, in1=wv4, op=op.mult)

                # reduce taps: tx then ty
                rt = mp.tile([P, JS * 2 * C], f16, name='rt', tag='rt')
                rv = rt[:].rearrange('p (j ty c) -> p j ty c', j=JS, ty=2, c=C)
                nc.vector.tensor_tensor(out=rv, in0=mv4[:, :, :, 0],
                                        in1=mv4[:, :, :, 1], op=op.add)
                nc.vector.tensor_tensor(out=rv, in0=rv, in1=mv4[:, :, :, 2],
                                        op=op.add)
                ot = outp.tile([P, C * JS], f32, name='ot', tag='o')
                o3 = ot[:].rearrange('p (c j) -> p c j', c=C, j=JS)
                r_ty0 = rt[:].rearrange('p (j ty c) -> p ty c j',
                                        j=JS, ty=2, c=C)
                nc.vector.tensor_tensor(out=o3, in0=r_ty0[:, 0], in1=r_ty0[:, 1],
                                        op=op.add)
                nc.sync.dma_start(out=feats_out[ix], in_=ot[:])

            ip_ctx.__exit__(None, None, None)
            outp_ctx.__exit__(None, None, None)
            mp_ctx.__exit__(None, None, None)
            gp_ctx.__exit__(None, None, None)

    nc.finalize()
    return nc


def _get_built(debug=False):
    global _built
    if _built is None or _built[1] != debug:
        _built = (_build(debug=debug), debug)
    return _built[0]


def kernel(grid_size, T_0w, center, pitch, images, transformations, T_cw,
           _debug=False, _trace=False):
    import sys
    if '/opt/trn_rl_repo' not in sys.path:
        sys.path.insert(0, '/opt/trn_rl_repo')
    from concourse.bass_utils import run_bass_kernel_spmd

    assert int(grid_size) == G
    images = np.asarray(images, np.float32)
    assert images.shape == (NCAM, C, H, W)

    imgb = _pack_images(images)
    in_maps = []
    for core in range(NCORES):
        cam, h = core // 2, core % 2
        in_maps.append({
            'imgb': imgb[cam],
            'params': _core_params(cam, h, T_0w, center, pitch,
                                   transformations, T_cw),
        })

    nc = _get_built(debug=_debug)
    try:
        res = run_bass_kernel_spmd(nc, in_maps, core_ids=list(range(NCORES)),
                                   trace=_trace)
    except ModuleNotFoundError:
        # no NTFF profile hook in this environment; run untraced
        res = run_bass_kernel_spmd(nc, in_maps, core_ids=list(range(NCORES)))
    kernel.last_results = res

    # ---- host assembly; in-slab voxel v = j*128 + p ----
    cams = [None] * NCAM
    for core in range(NCORES):
        cam, h = core // 2, core % 2
        r = res.results[core]
        f = r['feats'].reshape(HALF, P, C, JS).transpose(2, 0, 3, 1)
        f = f.reshape(C, HALF, G, G)          # [c, ix_l, iy, iz]
        e = r['extras'].reshape(5, P, HALF, JS).transpose(0, 2, 3, 1)
        e = e.reshape(5, HALF, G, G)
        half = np.concatenate([f, e], axis=0)  # [37, HALF, 64, 64]
        if cams[cam] is None:
            cams[cam] = [None, None]
        cams[cam][h] = half
    full = [np.concatenate(cams[cam], axis=1) for cam in range(NCAM)]

    perm = _get_perm()
    out = np.empty((NCAM * (C + 5), G, G, G), np.float32)
    for i in range(NCAM):
        cam = int(perm[i])
        out[i * C:(i + 1) * C] = full[cam][:C]
        out[NCAM * C + i] = full[cam][C]
        out[NCAM * C + NCAM + i] = full[cam][C + 1]
        out[NCAM * C + 2 * NCAM + 3 * i: NCAM * C + 2 * NCAM + 3 * i + 3] = \
            full[cam][C + 2:C + 5]
    return out
